# revision 1
# baseline (speedup 1.0000x reference)
"""IoU loss kernel for Trainium2, data-parallel over 8 NeuronCores.

Math (per box, columns = x-center, y-center, half-size s):
    w = relu(min(x+s, x'+s') - max(x-s, x'-s'))
      = relu((s+s') - max(|x-x'|, |s-s'|))          # S - max identity
    h likewise with y.
    overlap = w*h
    union   = 4s^2 + 4s'^2 - overlap = 2(S^2 + D^2) - overlap,
              S = s+s', D = s-s'
    iou     = overlap / (union + 1e-7)
    loss    = -sum(log(iou + 1e-7));  iou_sum = sum(iou)

Engine split per 128x1024-box tile:
  DVE   : dx, dy, S, D (fp32 strided reads -> fp16), abs_max fusions,
          subs, relus (tensor_scalar, 4x), overlap, union,
          tensor_tensor_reduce for iou (+ per-tile iou partial sum)
  ACT   : squares via Square(sqrt2 * x), 1/(u+eps) via Exp(-Ln(u+eps)),
          final Ln(iou+eps) with accum_out giving the loss partial sum.
          All functions live in the natural_log_exp_and_others table set.
  Host  : final [128, 2T] x 8 cores partial-sum reduction in float64.
"""

import numpy as np

import concourse.bass as bass
import concourse.mybir as mybir
from concourse import tile
from concourse.bass_utils import run_bass_kernel_spmd

N = 8388608
NCORES = 8
NS = N // NCORES  # 1048576 boxes per core
P = 128
W = 1024          # boxes per partition per tile
T = NS // (P * W)  # 8 tiles per core
EPS = 1e-7
RT2 = 1.4142135623730951

F32 = mybir.dt.float32
F16 = mybir.dt.float16
Op = mybir.AluOpType
Act = mybir.ActivationFunctionType


def _build(T_: int = T, W_: int = W, compile_passes: bool = True) -> bass.Bass:
    from concourse import bacc

    ns = P * W_ * T_
    nc = bacc.Bacc()
    outs_d = nc.dram_tensor("outputs", [ns, 3], F32, kind="ExternalInput")
    tars_d = nc.dram_tensor("targets", [ns, 3], F32, kind="ExternalInput")
    acc_d = nc.dram_tensor("acc", [P, 2 * T_], F32, kind="ExternalOutput")

    outs_v = outs_d[:, :].rearrange("(t p w) c -> t p (w c)", t=T_, p=P, w=W_)
    tars_v = tars_d[:, :].rearrange("(t p w) c -> t p (w c)", t=T_, p=P, w=W_)
    T, W = T_, W_

    from concourse.tile_rust import add_dep_helper

    with tile.TileContext(nc) as tc:
        with tc.tile_pool(name="main", bufs=2) as pool:
            accs = pool.tile([P, 2 * T], F32, tag="accs", bufs=1)
            eps_t = pool.tile([P, 1], F32, tag="eps", bufs=1)
            nc.vector.memset(eps_t[:, :], EPS)
            last_ttr = None
            RAWBUFS = 4
            ttr_hist: list = []
            dmaO_hist: list = []
            dmaT_hist: list = []
            for t in range(T):
                rawO = pool.tile([P, 3 * W], F32, tag="rawO", bufs=RAWBUFS)
                rawT = pool.tile([P, 3 * W], F32, tag="rawT", bufs=RAWBUFS)
                if t >= RAWBUFS:
                    # DMA instructions have a single sync-wait slot, but a
                    # slot-recycling load needs a WAR wait (slot readers, DVE
                    # sem), a WAW wait, and a lane-reuse wait. With bufs=4 and
                    # 2 DMAs/tile the recycled slot's DMA sits exactly 8 DMAs
                    # back, so WAW and lane-reuse share one semaphore; a
                    # Drain (large wait budget) on the SP sequencer absorbs
                    # all conditions onto the SP-seq clock, leaving the big
                    # loads wait-free.
                    dr = nc.sync.drain(fusable=False)
                    add_dep_helper(dr.ins, ttr_hist[t - RAWBUFS].ins, sync=True,
                                   reason="absorb DVE WAR tick")
                    add_dep_helper(dr.ins, dmaO_hist[t - RAWBUFS].ins, sync=True,
                                   reason="absorb old rawO DMA lane")
                    add_dep_helper(dr.ins, dmaT_hist[t - RAWBUFS].ins, sync=True,
                                   reason="absorb old rawT DMA lane")
                dmaO_hist.append(nc.sync.dma_start(out=rawO[:, :], in_=outs_v[t]))
                dmaT_hist.append(nc.sync.dma_start(out=rawT[:, :], in_=tars_v[t]))
                # The TT ISA struct has a single sync-wait slot, but dx below
                # depends on BOTH input DMAs. Absorb rawT's semaphore with a
                # tiny copy so dx only needs the rawO wait.
                dummy = pool.tile([P, 1], F32, tag="dummy")
                nc.vector.tensor_copy(dummy[:, :], rawT[:, 0:1])
                o3 = rawO.rearrange("p (w c) -> p w c", c=3)
                t3 = rawT.rearrange("p (w c) -> p w c", c=3)
                x1, y1, s1 = o3[:, :, 0], o3[:, :, 1], o3[:, :, 2]
                x2, y2, s2 = t3[:, :, 0], t3[:, :, 1], t3[:, :, 2]

                dx = pool.tile([P, W], F16, tag="dx")
                nc.vector.tensor_tensor(dx[:, :], x1, x2, Op.subtract)
                dy = pool.tile([P, W], F16, tag="dy")
                nc.vector.tensor_tensor(dy[:, :], y1, y2, Op.subtract)
                S = pool.tile([P, W], F16, tag="S")
                nc.vector.tensor_tensor(S[:, :], s1, s2, Op.add)
                D = pool.tile([P, W], F16, tag="D")
                nc.vector.tensor_tensor(D[:, :], s1, s2, Op.subtract)

                # |dx|, |dy|, |D| on the scalar engine (abs_max is CoreSim-only)
                adx = pool.tile([P, W], F16, tag="adx")
                nc.scalar.activation(adx[:, :], dx[:, :], Act.Abs)
                ady = pool.tile([P, W], F16, tag="ady")
                nc.scalar.activation(ady[:, :], dy[:, :], Act.Abs)
                aD = pool.tile([P, W], F16, tag="aD")
                nc.scalar.activation(aD[:, :], D[:, :], Act.Abs)

                mw = pool.tile([P, W], F16, tag="mw")
                nc.vector.tensor_tensor(mw[:, :], adx[:, :], aD[:, :], Op.max)
                mh = pool.tile([P, W], F16, tag="mh")
                nc.vector.tensor_tensor(mh[:, :], ady[:, :], aD[:, :], Op.max)

                wr = pool.tile([P, W], F16, tag="wr")
                nc.vector.tensor_sub(wr[:, :], S[:, :], mw[:, :])
                hr = pool.tile([P, W], F16, tag="hr")
                nc.vector.tensor_sub(hr[:, :], S[:, :], mh[:, :])

                w_ = pool.tile([P, W], F16, tag="w_")
                nc.vector.tensor_scalar_max(w_[:, :], wr[:, :], 0.0)
                h_ = pool.tile([P, W], F16, tag="h_")
                nc.vector.tensor_scalar_max(h_[:, :], hr[:, :], 0.0)

                ov = pool.tile([P, W], F16, tag="ov")
                nc.vector.tensor_mul(ov[:, :], w_[:, :], h_[:, :])

                # 2*S^2 and 2*D^2 on the scalar engine: Square(sqrt2 * x)
                qS = pool.tile([P, W], F16, tag="qS")
                nc.scalar.activation(qS[:, :], S[:, :], Act.Square, scale=RT2)
                qD = pool.tile([P, W], F16, tag="qD")
                nc.scalar.activation(qD[:, :], D[:, :], Act.Square, scale=RT2)
                qs = pool.tile([P, W], F16, tag="qs")
                nc.vector.tensor_add(qs[:, :], qS[:, :], qD[:, :])

                ue = pool.tile([P, W], F16, tag="ue")
                nc.vector.tensor_sub(ue[:, :], qs[:, :], ov[:, :])

                # r = 1/(ue + eps) = exp(-ln(ue + eps)); fp32 (can reach 1e7)
                lnu = pool.tile([P, W], F32, tag="lnu")
                nc.scalar.activation(lnu[:, :], ue[:, :], Act.Ln, bias=eps_t[:, 0:1])
                r = pool.tile([P, W], F32, tag="r")
                nc.scalar.activation(r[:, :], lnu[:, :], Act.Exp, scale=-1.0)

                # iou = overlap * r, with running per-partition sum into accs[:, t]
                iou = pool.tile([P, W], F16, tag="iou")
                nc.vector.tensor_mul(iou[:, :], ov[:, :], r[:, :])
                last_ttr = nc.vector.tensor_reduce(
                    accs[:, t : t + 1], iou[:, :], mybir.AxisListType.X, Op.add
                )

                # loss partial: sum of Ln(iou + eps) via activation accumulate
                li = pool.tile([P, W], F32, tag="li")
                last_act = nc.scalar.activation(
                    li[:, :],
                    iou[:, :],
                    Act.Ln,
                    bias=eps_t[:, 0:1],
                    accum_out=accs[:, T + t : T + t + 1],
                )
                ttr_hist.append(last_ttr)

            # acc store would need waits on both the DVE (iou accums) and ACT
            # (loss accums) sems; absorb both on an SP drain first.
            dr = nc.sync.drain(fusable=False)
            add_dep_helper(dr.ins, last_ttr.ins, sync=True,
                           reason="absorb DVE accum tick before acc store")
            add_dep_helper(dr.ins, last_act.ins, sync=True,
                           reason="absorb ACT accum tick before acc store")
            nc.sync.dma_start(out=acc_d[:, :], in_=accs[:, :])

    if compile_passes:
        # Bacc.compile runs generate_event_semaphores (splits multi-wait
        # instructions to satisfy the 1-wait-per-instruction HW limit),
        # extended-inst lowering, and ACT table loads.
        nc.compile()
    return nc


_NC_CACHE: list[bass.Bass] = []


def _get_nc() -> bass.Bass:
    if not _NC_CACHE:
        _NC_CACHE.append(_build())
    return _NC_CACHE[0]


def _run(inputs: dict, trace: bool = False, trace_kwargs: dict | None = None):
    outputs = np.ascontiguousarray(np.asarray(inputs["outputs"], dtype=np.float32))
    targets = np.ascontiguousarray(np.asarray(inputs["targets"], dtype=np.float32))
    assert outputs.shape == (N, 3) and targets.shape == (N, 3)

    nc = _get_nc()
    in_maps = [
        {
            "outputs": outputs[c * NS : (c + 1) * NS],
            "targets": targets[c * NS : (c + 1) * NS],
        }
        for c in range(NCORES)
    ]
    kw = {}
    if trace:
        kw["trace"] = True
        if trace_kwargs:
            kw["trace_kwargs"] = trace_kwargs
    res = run_bass_kernel_spmd(nc, in_maps, list(range(NCORES)), **kw)

    iou_sum = 0.0
    loss = 0.0
    for c in range(NCORES):
        acc = np.asarray(res.results[c]["acc"], dtype=np.float64)
        iou_sum += acc[:, :T].sum()
        loss += acc[:, T:].sum()
    loss = -loss
    return (np.float32(loss), np.float32(iou_sum)), res


def kernel(**inputs) -> tuple:
    (loss, iou_sum), _ = _run(inputs)
    return (loss, iou_sum)



# revision 6
# speedup vs baseline: 1.4729x; 1.4729x over previous
"""IoU loss kernel for Trainium2, data-parallel over 8 NeuronCores.

Math (per box, columns = x-center, y-center, half-size s):
    w  = relu(S - max(|dx|, |D|)),  S = s1+s2, D = s1-s2, dx = x1-x2
    h  = relu(S - max(|dy|, |D|))
    ov = w*h
    ue = 2S^2 + 2D^2 - ov                      (union)
    iou = ov / (ue + 1e-7)
    loss = -sum(log(iou + 1e-7));  iou_sum = sum(iou)

Design (per 128x1024-box tile, W=1024):
  DVE:  one custom ABS_DIFF op |O-T| over the full interleaved stream with a
        deinterleaving output AP -> [|dx| | |dy| | |D|] contiguous blocks
        (kills the whole separate abs stage), S into the adjacent block,
        then five f16 2x tensor_tensor ops: max / sub (2W fused via
        broadcast APs), ov, qs, ue, d = ln(ov+c) - ln(ue+eps).
  ACT:  Relu(w~|h~), Square(sqrt2*[|D| | S]) -> [2D^2|2S^2],
        Ln(ue+eps), Ln(ov+1e-9),
        Exp(d)  + accum  -> per-tile  sum(iou)       (free reduction)
        Relu(d - ln eps) + accum -> per-tile loss partial; the clamp
        max(d, ln eps) == ln(iou+eps) up to O(1e-3) relative on the sums.
  Host: sum the [128, 2T] partials of 8 cores in float64;
        loss = -(sum_relu + N*ln(eps)).
"""

import numpy as np

import concourse.bass as bass
import concourse.mybir as mybir
import concourse.dve_ops as dve_ops
from concourse import tile
from concourse.bass_utils import run_bass_kernel_spmd
from concourse.dve_spec import Spec, Src0, Src1, maxx, lower as dve_lower
from concourse.dve_uop import DveOpSpec

N = 8388608
NCORES = 8
NS = N // NCORES      # 1048576 boxes per core
P = 128
W = 1024              # boxes per partition per tile
T = NS // (P * W)     # 8 tiles per core
EPS = 1e-7
CLO = 1e-9            # bias for Ln(ov + .): << eps*typical(ue)
LNEPS = float(np.log(np.float32(EPS)))
RT2 = 1.4142135623730951

F32 = mybir.dt.float32
F16 = mybir.dt.float16
Op = mybir.AluOpType
Act = mybir.ActivationFunctionType


def _register_absdiff() -> "dve_ops.DveOp":
    """Register |Src0 - Src1| as a custom DVE op (row past the builtin 16).

    One 1x instruction handles the fp32->f16 abs-diff of the whole
    interleaved (w c) stream; the deinterleaving is free via the output
    access pattern."""
    name = "ABS_DIFF_ANT_K"
    for o in dve_ops.OPS:
        if o.name == name:
            return o
    spec = Spec(
        body=maxx(Src0 - Src1, Src1 - Src0),
        reference=lambda in0, in1, s0, s1, imm2: np.maximum(
            in0.astype(np.float32) - in1.astype(np.float32),
            in1.astype(np.float32) - in0.astype(np.float32),
        ),
    )
    row = max(dve_ops._SUB_OPCODE_FOR_NAME.values()) + 1
    assert row < 0x20, "custom DVE opcode rows exhausted"
    shas = {}
    for ver in ("v3", "v4"):
        uops = dve_lower(spec, ver=ver)
        shas[ver] = DveOpSpec(name=name, opcode=row, uops=uops, rd1_en=True).sha(ver)
    op = dve_ops.DveOp(name, spec, False, shas)
    dve_ops.OPS.append(op)
    dve_ops.CUSTOM_DVE_SPECS[name] = spec
    dve_ops._SUB_OPCODE_FOR_NAME[name] = row
    return op


def _build(T_: int = T, W_: int = W, raw_bufs: int = 3,
           compile_passes: bool = True, trace_sim: bool = False) -> bass.Bass:
    from concourse import bacc

    absdiff = _register_absdiff()
    T, W = T_, W_
    ns = P * W * T
    nc = bacc.Bacc()
    outs_d = nc.dram_tensor("outputs", [ns, 3], F32, kind="ExternalInput")
    tars_d = nc.dram_tensor("targets", [ns, 3], F32, kind="ExternalInput")
    acc_d = nc.dram_tensor("acc", [P, 2 * T], F32, kind="ExternalOutput")

    # Each DMA spans SPAN compute tiles (24KB/partition contiguous): fewer,
    # larger transfers amortize the per-DMA DGE/sem overhead.
    SPAN = 2
    assert T % SPAN == 0
    outs_v = outs_d[:, :].rearrange("(g p w) c -> g p (w c)", g=T // SPAN, p=P)
    tars_v = tars_d[:, :].rearrange("(g p w) c -> g p (w c)", g=T // SPAN, p=P)

    with tile.TileContext(nc, trace_sim=trace_sim) as tc:
        with tc.tile_pool(name="main", bufs=2) as pool:
            eps_t = pool.tile([P, 1], F32, tag="eps", bufs=1)
            nc.vector.memset(eps_t[:, :], EPS)
            clo_t = pool.tile([P, 1], F32, tag="clo", bufs=1)
            nc.vector.memset(clo_t[:, :], CLO)
            nlc_t = pool.tile([P, 1], F32, tag="nlc", bufs=1)
            nc.vector.memset(nlc_t[:, :], -LNEPS)
            accs = pool.tile([P, 2 * T], F32, tag="accs", bufs=1)

            for t in range(T):
                g, sub = divmod(t, SPAN)
                if sub == 0:
                    rawO = pool.tile([P, SPAN * 3 * W], F32, tag="rawO",
                                     bufs=raw_bufs)
                    rawT = pool.tile([P, SPAN * 3 * W], F32, tag="rawT",
                                     bufs=raw_bufs)
                    if g == 0:
                        # Prologue: per-tile DMA halves so the first compute
                        # tile starts after ~2 quarter transfers, not 2 full
                        # span transfers.
                        for s in range(SPAN):
                            c0, c1 = s * 3 * W, (s + 1) * 3 * W
                            nc.sync.dma_start(
                                out=rawO[:, c0:c1], in_=outs_v[g][:, c0:c1]
                            )
                            nc.sync.dma_start(
                                out=rawT[:, c0:c1], in_=tars_v[g][:, c0:c1]
                            )
                    else:
                        nc.sync.dma_start(out=rawO[:, :], in_=outs_v[g])
                        nc.sync.dma_start(out=rawT[:, :], in_=tars_v[g])

                o3 = rawO[:, sub * 3 * W : (sub + 1) * 3 * W].rearrange(
                    "p (w c) -> p w c", c=3
                )
                t3 = rawT[:, sub * 3 * W : (sub + 1) * 3 * W].rearrange(
                    "p (w c) -> p w c", c=3
                )

                # AD3 blocks: [ |dx| | |dy| | |D| | S ]
                AD3 = pool.tile([P, 4 * W], F16, tag="AD3")
                ad4 = AD3.rearrange("p (c w) -> p c w", c=4)
                nc.vector._custom_dve(
                    absdiff,
                    out=ad4[:, 0:3, :],
                    in0=o3.transpose([0, 2, 1]),
                    in1=t3.transpose([0, 2, 1]),
                )
                nc.vector.tensor_tensor(ad4[:, 3, :], o3[:, :, 2], t3[:, :, 2], Op.add)

                # mwmh = max([|dx| | |dy|], |D| broadcast)     [P, 2, W]
                MM = pool.tile([P, 2 * W], F16, tag="MM")
                mm2 = MM.rearrange("p (c w) -> p c w", c=2)
                aD_rep = AD3[:, 2 * W : 3 * W].unsqueeze(1).broadcast_to([P, 2, W])
                nc.vector.tensor_tensor(mm2[:, :, :], ad4[:, 0:2, :], aD_rep, Op.max)

                # [w~ | h~] = S broadcast - mwmh
                WT = pool.tile([P, 2 * W], F16, tag="WT")
                wt2 = WT.rearrange("p (c w) -> p c w", c=2)
                S_rep = AD3[:, 3 * W : 4 * W].unsqueeze(1).broadcast_to([P, 2, W])
                nc.vector.tensor_tensor(wt2[:, :, :], S_rep, mm2[:, :, :], Op.subtract)

                # RW = relu([w~ | h~]) on ACT
                RW = pool.tile([P, 2 * W], F16, tag="RW")
                nc.scalar.activation(RW[:, :], WT[:, :], Act.Relu)

                # ov = w * h
                OV = pool.tile([P, W], F16, tag="OV")
                nc.vector.tensor_tensor(OV[:, :], RW[:, 0:W], RW[:, W : 2 * W], Op.mult)

                # QQ = Square(sqrt2 * [|D| | S]) = [2D^2 | 2S^2]
                QQ = pool.tile([P, 2 * W], F16, tag="QQ")
                nc.scalar.activation(
                    QQ[:, :], AD3[:, 2 * W : 4 * W], Act.Square, scale=RT2
                )
                QS = pool.tile([P, W], F16, tag="QS")
                nc.vector.tensor_tensor(QS[:, :], QQ[:, 0:W], QQ[:, W : 2 * W], Op.add)

                UE = pool.tile([P, W], F16, tag="UE")
                nc.vector.tensor_tensor(UE[:, :], QS[:, :], OV[:, :], Op.subtract)

                # LL = [Ln(ue+eps) | Ln(ov+c)]
                LL = pool.tile([P, 2 * W], F16, tag="LL")
                nc.scalar.activation(LL[:, 0:W], UE[:, :], Act.Ln, bias=eps_t[:, 0:1])
                nc.scalar.activation(
                    LL[:, W : 2 * W], OV[:, :], Act.Ln, bias=clo_t[:, 0:1]
                )

                # d = ln(ov+c) - ln(ue+eps)
                DD = pool.tile([P, W], F16, tag="DD")
                nc.vector.tensor_tensor(
                    DD[:, :], LL[:, W : 2 * W], LL[:, 0:W], Op.subtract
                )

                # iou = Exp(d); accum -> sum(iou) partial
                IOU = pool.tile([P, W], F16, tag="IOU")
                nc.scalar.activation(
                    IOU[:, :], DD[:, :], Act.Exp, accum_out=accs[:, t : t + 1]
                )
                # Relu(d - ln eps); accum -> sum(max(d, ln eps)) - W*ln(eps)
                LR = pool.tile([P, W], F16, tag="LR")
                nc.scalar.activation(
                    LR[:, :],
                    DD[:, :],
                    Act.Relu,
                    bias=nlc_t[:, 0:1],
                    accum_out=accs[:, T + t : T + t + 1],
                )

            nc.sync.dma_start(out=acc_d[:, :], in_=accs[:, :])

    if compile_passes:
        nc.compile()
    return nc


_NC_CACHE: list[bass.Bass] = []


def _get_nc() -> bass.Bass:
    if not _NC_CACHE:
        _NC_CACHE.append(_build())
    return _NC_CACHE[0]


def _reduce_host(results) -> tuple:
    iou_sum = 0.0
    loss_relu = 0.0
    for c in range(NCORES):
        acc = np.asarray(results[c]["acc"], dtype=np.float64)
        iou_sum += acc[:, :T].sum()
        loss_relu += acc[:, T:].sum()
    loss = -(loss_relu + N * LNEPS)
    return (np.float32(loss), np.float32(iou_sum))


def _run(inputs: dict, trace: bool = False, trace_kwargs: dict | None = None):
    outputs = np.ascontiguousarray(np.asarray(inputs["outputs"], dtype=np.float32))
    targets = np.ascontiguousarray(np.asarray(inputs["targets"], dtype=np.float32))
    assert outputs.shape == (N, 3) and targets.shape == (N, 3)

    nc = _get_nc()
    in_maps = [
        {
            "outputs": outputs[c * NS : (c + 1) * NS],
            "targets": targets[c * NS : (c + 1) * NS],
        }
        for c in range(NCORES)
    ]
    kw = {}
    if trace:
        kw["trace"] = True
        if trace_kwargs:
            kw["trace_kwargs"] = trace_kwargs
    res = run_bass_kernel_spmd(nc, in_maps, list(range(NCORES)), **kw)
    return _reduce_host(res.results), res


def kernel(**inputs) -> tuple:
    (loss, iou_sum), _ = _run(inputs)
    return (loss, iou_sum)


# revision 8
# speedup vs baseline: 225.9150x; 153.3792x over previous
"""IoU loss kernel for Trainium2, data-parallel over 8 NeuronCores.

Math (per box, columns = x-center, y-center, half-size s):
    w  = relu(S - max(|dx|, |D|)),  S = s1+s2, D = s1-s2, dx = x1-x2
    h  = relu(S - max(|dy|, |D|))
    ov = w*h
    ue = 2S^2 + 2D^2 - ov                      (union)
    iou = ov / (ue + 1e-7)
    loss = -sum(log(iou + 1e-7));  iou_sum = sum(iou)

Design (per 128x1024-box tile, W=1024):
  DVE:  one custom ABS_DIFF op |O-T| over the full interleaved stream with a
        deinterleaving output AP -> [|dx| | |dy| | |D|] contiguous blocks
        (kills the whole separate abs stage), S into the adjacent block,
        then five f16 2x tensor_tensor ops: max / sub (2W fused via
        broadcast APs), ov, qs, ue, d = ln(ov+c) - ln(ue+eps).
  ACT:  Relu(w~|h~), Square(sqrt2*[|D| | S]) -> [2D^2|2S^2],
        Ln(ue+eps), Ln(ov+1e-9),
        Exp(d)  + accum  -> per-tile  sum(iou)       (free reduction)
        Relu(d - ln eps) + accum -> per-tile loss partial; the clamp
        max(d, ln eps) == ln(iou+eps) up to O(1e-3) relative on the sums.
  Host: sum the [128, 2T] partials of 8 cores in float64;
        loss = -(sum_relu + N*ln(eps)).
"""

import numpy as np

import concourse.bass as bass
import concourse.mybir as mybir
import concourse.dve_ops as dve_ops
from concourse import tile
from concourse.bass_utils import run_bass_kernel_spmd
from concourse.dve_spec import Spec, Src0, Src1, maxx, lower as dve_lower
from concourse.dve_uop import DveOpSpec

N = 8388608
NCORES = 8
NS = N // NCORES      # 1048576 boxes per core
P = 128
W = 1024              # boxes per partition per tile
T = NS // (P * W)     # 8 tiles per core
EPS = 1e-7
CLO = 1e-9            # bias for Ln(ov + .): << eps*typical(ue)
LNEPS = float(np.log(np.float32(EPS)))
RT2 = 1.4142135623730951

F32 = mybir.dt.float32
F16 = mybir.dt.float16
Op = mybir.AluOpType
Act = mybir.ActivationFunctionType


def _register_absdiff() -> "dve_ops.DveOp":
    """Register |Src0 - Src1| as a custom DVE op (row past the builtin 16).

    One 1x instruction handles the fp32->f16 abs-diff of the whole
    interleaved (w c) stream; the deinterleaving is free via the output
    access pattern."""
    name = "ABS_DIFF_ANT_K"
    for o in dve_ops.OPS:
        if o.name == name:
            return o
    spec = Spec(
        body=maxx(Src0 - Src1, Src1 - Src0),
        reference=lambda in0, in1, s0, s1, imm2: np.maximum(
            in0.astype(np.float32) - in1.astype(np.float32),
            in1.astype(np.float32) - in0.astype(np.float32),
        ),
    )
    row = max(dve_ops._SUB_OPCODE_FOR_NAME.values()) + 1
    assert row < 0x20, "custom DVE opcode rows exhausted"
    shas = {}
    for ver in ("v3", "v4"):
        uops = dve_lower(spec, ver=ver)
        shas[ver] = DveOpSpec(name=name, opcode=row, uops=uops, rd1_en=True).sha(ver)
    op = dve_ops.DveOp(name, spec, False, shas)
    dve_ops.OPS.append(op)
    dve_ops.CUSTOM_DVE_SPECS[name] = spec
    dve_ops._SUB_OPCODE_FOR_NAME[name] = row
    return op


def _build(T_: int = T, W_: int = W, raw_bufs: int = 3,
           compile_passes: bool = True, trace_sim: bool = False) -> bass.Bass:
    from concourse import bacc

    absdiff = _register_absdiff()
    T, W = T_, W_
    ns = P * W * T
    nc = bacc.Bacc()
    outs_d = nc.dram_tensor("outputs", [ns, 3], F32, kind="ExternalInput")
    tars_d = nc.dram_tensor("targets", [ns, 3], F32, kind="ExternalInput")
    acc_d = nc.dram_tensor("acc", [P, 2 * T], F32, kind="ExternalOutput")

    # Each DMA spans SPAN compute tiles (24KB/partition contiguous): fewer,
    # larger transfers amortize the per-DMA DGE/sem overhead.
    SPAN = 2
    assert T % SPAN == 0
    outs_v = outs_d[:, :].rearrange("(g p w) c -> g p (w c)", g=T // SPAN, p=P)
    tars_v = tars_d[:, :].rearrange("(g p w) c -> g p (w c)", g=T // SPAN, p=P)

    with tile.TileContext(nc, trace_sim=trace_sim) as tc:
        with tc.tile_pool(name="main", bufs=2) as pool:
            eps_t = pool.tile([P, 1], F32, tag="eps", bufs=1)
            nc.vector.memset(eps_t[:, :], EPS)
            clo_t = pool.tile([P, 1], F32, tag="clo", bufs=1)
            nc.vector.memset(clo_t[:, :], CLO)
            nlc_t = pool.tile([P, 1], F32, tag="nlc", bufs=1)
            nc.vector.memset(nlc_t[:, :], -LNEPS)
            accs = pool.tile([P, 2 * T], F32, tag="accs", bufs=1)

            for t in range(T):
                g, sub = divmod(t, SPAN)
                if sub == 0:
                    rawO = pool.tile([P, SPAN * 3 * W], F32, tag="rawO",
                                     bufs=raw_bufs)
                    rawT = pool.tile([P, SPAN * 3 * W], F32, tag="rawT",
                                     bufs=raw_bufs)
                    if g == 0:
                        # Prologue: per-tile DMA halves so the first compute
                        # tile starts after ~2 quarter transfers, not 2 full
                        # span transfers.
                        for s in range(SPAN):
                            c0, c1 = s * 3 * W, (s + 1) * 3 * W
                            nc.sync.dma_start(
                                out=rawO[:, c0:c1], in_=outs_v[g][:, c0:c1]
                            )
                            nc.sync.dma_start(
                                out=rawT[:, c0:c1], in_=tars_v[g][:, c0:c1]
                            )
                    else:
                        nc.sync.dma_start(out=rawO[:, :], in_=outs_v[g])
                        nc.sync.dma_start(out=rawT[:, :], in_=tars_v[g])

                o3 = rawO[:, sub * 3 * W : (sub + 1) * 3 * W].rearrange(
                    "p (w c) -> p w c", c=3
                )
                t3 = rawT[:, sub * 3 * W : (sub + 1) * 3 * W].rearrange(
                    "p (w c) -> p w c", c=3
                )

                # AD3 blocks: [ |dx| | |dy| | |D| | S ]
                AD3 = pool.tile([P, 4 * W], F16, tag="AD3")
                ad4 = AD3.rearrange("p (c w) -> p c w", c=4)
                nc.vector._custom_dve(
                    absdiff,
                    out=ad4[:, 0:3, :],
                    in0=o3.transpose([0, 2, 1]),
                    in1=t3.transpose([0, 2, 1]),
                )
                nc.vector.tensor_tensor(ad4[:, 3, :], o3[:, :, 2], t3[:, :, 2], Op.add)

                # mwmh = max([|dx| | |dy|], |D| broadcast)     [P, 2, W]
                MM = pool.tile([P, 2 * W], F16, tag="MM", bufs=1)
                mm2 = MM.rearrange("p (c w) -> p c w", c=2)
                aD_rep = AD3[:, 2 * W : 3 * W].unsqueeze(1).broadcast_to([P, 2, W])
                nc.vector.tensor_tensor(mm2[:, :, :], ad4[:, 0:2, :], aD_rep, Op.max)

                # [w~ | h~] = S broadcast - mwmh
                WT = pool.tile([P, 2 * W], F16, tag="WT")
                wt2 = WT.rearrange("p (c w) -> p c w", c=2)
                S_rep = AD3[:, 3 * W : 4 * W].unsqueeze(1).broadcast_to([P, 2, W])
                nc.vector.tensor_tensor(wt2[:, :, :], S_rep, mm2[:, :, :], Op.subtract)

                # relu([w~ | h~]) in place on ACT
                nc.scalar.activation(WT[:, :], WT[:, :], Act.Relu)

                # ov = w * h
                OV = pool.tile([P, W], F16, tag="OV")
                nc.vector.tensor_tensor(OV[:, :], WT[:, 0:W], WT[:, W : 2 * W], Op.mult)

                # QQ = Square(sqrt2 * [|D| | S]) = [2D^2 | 2S^2]
                QQ = pool.tile([P, 2 * W], F16, tag="QQ")
                nc.scalar.activation(
                    QQ[:, :], AD3[:, 2 * W : 4 * W], Act.Square, scale=RT2
                )
                QS = pool.tile([P, W], F16, tag="QS", bufs=1)
                nc.vector.tensor_tensor(QS[:, :], QQ[:, 0:W], QQ[:, W : 2 * W], Op.add)

                UE = pool.tile([P, W], F16, tag="UE")
                nc.vector.tensor_tensor(UE[:, :], QS[:, :], OV[:, :], Op.subtract)

                # LL = [Ln(ue+eps) | Ln(ov+c)]
                LL = pool.tile([P, 2 * W], F16, tag="LL")
                nc.scalar.activation(LL[:, 0:W], UE[:, :], Act.Ln, bias=eps_t[:, 0:1])
                nc.scalar.activation(
                    LL[:, W : 2 * W], OV[:, :], Act.Ln, bias=clo_t[:, 0:1]
                )

                # d = ln(ov+c) - ln(ue+eps)
                DD = pool.tile([P, W], F16, tag="DD")
                nc.vector.tensor_tensor(
                    DD[:, :], LL[:, W : 2 * W], LL[:, 0:W], Op.subtract
                )

                # iou = Exp(d); accum -> sum(iou) partial
                IOU = pool.tile([P, W], F16, tag="IOU", bufs=1)
                nc.scalar.activation(
                    IOU[:, :], DD[:, :], Act.Exp, accum_out=accs[:, t : t + 1]
                )
                # Relu(d - ln eps); accum -> sum(max(d, ln eps)) - W*ln(eps)
                LR = pool.tile([P, W], F16, tag="LR", bufs=1)
                nc.scalar.activation(
                    LR[:, :],
                    DD[:, :],
                    Act.Relu,
                    bias=nlc_t[:, 0:1],
                    accum_out=accs[:, T + t : T + t + 1],
                )

            nc.sync.dma_start(out=acc_d[:, :], in_=accs[:, :])

    if compile_passes:
        nc.compile()
    return nc


_NC_CACHE: list[bass.Bass] = []


def _get_nc() -> bass.Bass:
    if not _NC_CACHE:
        _NC_CACHE.append(_build())
    return _NC_CACHE[0]


def _reduce_host(results) -> tuple:
    iou_sum = 0.0
    loss_relu = 0.0
    for c in range(NCORES):
        acc = np.asarray(results[c]["acc"], dtype=np.float64)
        iou_sum += acc[:, :T].sum()
        loss_relu += acc[:, T:].sum()
    loss = -(loss_relu + N * LNEPS)
    return (np.float32(loss), np.float32(iou_sum))


def _run(inputs: dict, trace: bool = False, trace_kwargs: dict | None = None):
    outputs = np.ascontiguousarray(np.asarray(inputs["outputs"], dtype=np.float32))
    targets = np.ascontiguousarray(np.asarray(inputs["targets"], dtype=np.float32))
    assert outputs.shape == (N, 3) and targets.shape == (N, 3)

    nc = _get_nc()
    in_maps = [
        {
            "outputs": outputs[c * NS : (c + 1) * NS],
            "targets": targets[c * NS : (c + 1) * NS],
        }
        for c in range(NCORES)
    ]
    kw = {}
    if trace:
        kw["trace"] = True
        if trace_kwargs:
            kw["trace_kwargs"] = trace_kwargs
    res = run_bass_kernel_spmd(nc, in_maps, list(range(NCORES)), **kw)
    return _reduce_host(res.results), res


def kernel(**inputs) -> tuple:
    (loss, iou_sum), _ = _run(inputs)
    return (loss, iou_sum)
